# revision 14
# baseline (speedup 1.0000x reference)
"""Trainium2 Bass kernel: causal multi-head attention with RoPE.

Model: B=4, L=2048, H=2048, NH=16 heads, head_dim=128.
  q = x @ Wq.T ; k = x @ Wk.T ; v = x @ Wv.T        (per-head split)
  q, k <- RoPE(q, k)
  attn = softmax(mask(q k^T / sqrt(hd)))
  out  = (attn @ v) heads-concat @ Wo.T

Sharding (8 cores): hybrid batch x tensor-parallel.  Core c handles
batch b = c//2 and heads half*8..half*8+7 with half = c%2.  Wq/Wk/Wv are
column-sharded (8 heads per core), Wo row-sharded; each core produces a
partial y[b] and the host sums the two partials per batch (the unshard
step) and concatenates batches.

Per-core dataflow (all SBUF-resident, bf16 inputs / fp32 accumulation):
  phase A: Q^T, K^T  [128d x 2048pos] per head (d-major) via PE matmuls
           with fused RoPE (rotate-half partition shuffle via SBUF->SBUF
           DMA, elementwise on DVE); V [128pos x 1024d] pos-major.
  phase B: flash-style causal attention per (head, 512-wide q chunk):
           S^T tile = K_blk^T Q_chunk (PE), P = exp(S^T/sqrt(d)) (ACT),
           block-sparse causal structure with a triangular-mask multiply
           on diagonal blocks (DVE); softmax denominators accumulate on
           DVE into a per-strip D tile + one ones-matmul per strip;
           reciprocal+broadcast+scale normalization (DVE + GPSIMD).
  phase C: y^T partial = Wo_shard O^T (PE) -> DRAM fp32.

Scheduling: the exp stream on ACT (~0.68us per 128x512 block) is slower
than the S+O matmuls it feeds (~0.42us PE), so attention alone is
ACT-bound.  The emission order therefore keeps PE filler next to every
exp burst: attention runs pair-major (q-chunks 0,1 for all heads, then
2,3); the second half of the V projection (kp blocks 8..15, only needed
by pair 2,3) interleaves into pair (0,1), and the out-projection tiles
for q-chunks 0,1 interleave into pair (2,3).

Host-side prep: weights and x are pre-permuted to partition-major
[128, a, m] layouts so every DMA is a dense per-partition pattern
(fast descriptor issue), and all tensors are pre-cast to bf16.
"""

import math
import numpy as np

B, L, H, NH, HD = 4, 2048, 2048, 16, 128
ROPE_BASE = 10000.0
NCORES = 8
HPC = 8          # heads per core
QC = 512         # q chunk width
NQC = L // QC    # 4 q chunks
NKB = L // 128   # 16 kp blocks
VC = 128         # v-phase x chunk width (1 kp block)
SCALE = 1.0 / math.sqrt(HD)

_cache = {}


def _analyze_mask(mask2d):
    """Classify each (q_block, kp_block) 128x128 block of the [L, L] mask.

    Returns (block_kind[16][16] with 0=empty,1=full,2=mixed, patterns,
    pattern_idx dict keyed by block coords). mask2d is int32 [L, L],
    rows=q, cols=kp.
    """
    nb = L // 128
    kind = [[0] * nb for _ in range(nb)]
    patterns = []
    pat_key_to_idx = {}
    block_pat = {}
    for qb in range(nb):
        rows = mask2d[qb * 128:(qb + 1) * 128]
        for kb in range(nb):
            blk = rows[:, kb * 128:(kb + 1) * 128]
            s = int(blk.sum())
            if s == 0:
                kind[qb][kb] = 0
            elif s == 128 * 128:
                kind[qb][kb] = 1
            else:
                kind[qb][kb] = 2
                key = blk.tobytes()
                idx = pat_key_to_idx.get(key)
                if idx is None:
                    idx = len(patterns)
                    pat_key_to_idx[key] = idx
                    # stored transposed: S^T tiles are [kp, q]
                    patterns.append(np.ascontiguousarray(blk.T))
                block_pat[(qb, kb)] = idx
    return kind, patterns, block_pat


def _build(kind, block_pat, n_patterns):
    """Build the SPMD bass program (same for all 8 cores)."""
    import concourse.bass as bass
    import concourse.bacc as bacc
    import concourse.mybir as mybir
    import concourse.tile as tile

    fp32 = mybir.dt.float32
    bf16 = mybir.dt.bfloat16
    EXP = mybir.ActivationFunctionType.Exp

    nc = bacc.Bacc("TRN2", target_bir_lowering=False, debug=False)

    NHC = H // 128  # 16 input-feature blocks

    # host-side pre-permuted, partition-major layouts
    xd = nc.dram_tensor("xd", [128, NHC, L], bf16, kind="ExternalInput")
    wqd = nc.dram_tensor("wqd", [128, NHC, HPC * HD], bf16,
                         kind="ExternalInput")
    wkd = nc.dram_tensor("wkd", [128, NHC, HPC * HD], bf16,
                         kind="ExternalInput")
    wvd = nc.dram_tensor("wvd", [128, NHC, HPC * HD], bf16,
                         kind="ExternalInput")
    wod = nc.dram_tensor("wod", [128, HPC, H], bf16, kind="ExternalInput")
    cosd = nc.dram_tensor("cosd", [HD, L], bf16, kind="ExternalInput")
    sinmd = nc.dram_tensor("sinmd", [HD, L], bf16, kind="ExternalInput")
    npat = max(n_patterns, 1)
    maskd = nc.dram_tensor("maskd", [npat, 128, 128], bf16,
                           kind="ExternalInput")
    yT = nc.dram_tensor("yT", [H, L], fp32, kind="ExternalOutput")

    def qk_phase(tc, w_dram, out_a, wpool, xpool, tpool, pspool, wtag,
                 cos_sb, sinm_sb, delay_after=None, gmajor_first=False):
        """Q^T / K^T d-major projection + fused RoPE per (head, chunk).

        delay_after: instructions the weight DMA must wait for -- used to
        keep the K-phase weight prefetch off the critical head-of-kernel
        DMA bandwidth.

        gmajor_first: for the head-of-kernel call, process j=0 in
        weight-group-major order (8 heads accumulate in 8 PSUM banks),
        interleaving the w/x group DMAs, so the first matmul only waits
        on 1.5MB of DMA instead of the full 6MB.
        """
        from concourse.tile import add_dep_helper
        w_sb = wpool.tile([128, NHC, HPC * HD], bf16, tag="w",
                          name=f"w_{wtag}")
        w_insts = []
        if not gmajor_first:
            for g in range(4):
                w_insts.append(
                    nc.sync.dma_start(out=w_sb[:, 4 * g:4 * g + 4, :],
                                      in_=w_dram[:, 4 * g:4 * g + 4, :]))
            if delay_after:
                for wi in w_insts:
                    for di in delay_after:
                        add_dep_helper(wi.ins, di.ins,
                                       reason="defer weight prefetch")

        def rope(q, js):
            # rotate-half: pure partition swap, done by SBUF->SBUF DMA
            rq = tpool.tile([128, QC], bf16, tag="rotq")
            nc.sync.dma_start(out=rq[0:64, :], in_=q[64:128, :])
            nc.sync.dma_start(out=rq[64:128, :], in_=q[0:64, :])
            nc.vector.tensor_mul(rq[:], rq[:], sinm_sb[:, js])
            nc.vector.tensor_mul(q, q, cos_sb[:, js])
            nc.vector.tensor_add(q, q, rq[:])

        x0_insts = []
        for j in range(NQC):
            js = slice(j * QC, (j + 1) * QC)
            x_sb = xpool.tile([128, NHC, QC], bf16, tag="xcols",
                              name=f"x_{wtag}{j}")
            if gmajor_first and j == 0:
                psd = [pspool.tile([128, QC], fp32, tag="ps_proj",
                                   name=f"ps0_{wtag}{h}")
                       for h in range(HPC)]
                for g in range(4):
                    w_insts.append(
                        nc.sync.dma_start(out=w_sb[:, 4 * g:4 * g + 4, :],
                                          in_=w_dram[:, 4 * g:4 * g + 4, :]))
                    x0_insts.append(
                        nc.sync.dma_start(out=x_sb[:, 4 * g:4 * g + 4, :],
                                          in_=xd[:, 4 * g:4 * g + 4, js]))
                    for h in range(HPC):
                        for hc in range(4 * g, 4 * g + 4):
                            nc.tensor.matmul(
                                psd[h][:],
                                w_sb[:, hc, h * HD:(h + 1) * HD],
                                x_sb[:, hc, :],
                                start=(hc == 0), stop=(hc == NHC - 1))
                for h in range(HPC):
                    q = out_a[:, h, js]
                    nc.scalar.copy(q, psd[h][:])
                    rope(q, js)
                continue
            for g in range(4):
                di = nc.sync.dma_start(out=x_sb[:, 4 * g:4 * g + 4, :],
                                       in_=xd[:, 4 * g:4 * g + 4, js])
                if j == 0:
                    x0_insts.append(di)
            for h in range(HPC):
                ps = pspool.tile([128, QC], fp32, tag="ps_proj")
                for hc in range(NHC):
                    nc.tensor.matmul(
                        ps[:],
                        w_sb[:, hc, h * HD:(h + 1) * HD],
                        x_sb[:, hc, :],
                        start=(hc == 0), stop=(hc == NHC - 1))
                q = out_a[:, h, js]
                nc.scalar.copy(q, ps[:])
                rope(q, js)
        return x0_insts

    with tile.TileContext(nc) as tc:
        with tc.tile_pool(name="persist", bufs=1, side="left") as persist:
            # one combined small-constant tile: [trimask patterns | ones]
            cst = persist.tile([128, npat * 128 + 128], bf16, tag="cst")
            for p in range(n_patterns):
                nc.scalar.dma_start(out=cst[:, p * 128:(p + 1) * 128],
                                    in_=maskd[p])
            ones_col = npat * 128
            nc.vector.memset(cst[:, ones_col:ones_col + 128], 1.0)
            # warm up the gpsimd partition_broadcast library during the
            # head-of-kernel DMA wait: the first broadcast triggers a
            # ~12us library load that would otherwise stall the in-order
            # PE stream mid-attention.
            warm = persist.tile([128, 16], bf16, tag="warm")
            nc.gpsimd.partition_broadcast(warm[:],
                                          cst[0:1, ones_col:ones_col + 16])
            QTa = persist.tile([HD, HPC, L], bf16, tag="qta")
            KTa = persist.tile([HD, HPC, L], bf16, tag="kta")

            # ---------------- phase A: Q/K projections + RoPE -------------
            # Manual pool lifetimes (non-LIFO): Q/K weights/x/rope tables
            # are freed before the V+attention section.
            wpool_cm = tc.tile_pool(name="wpool", bufs=2, side="right")
            wpool = wpool_cm.__enter__()
            ropec_cm = tc.tile_pool(name="ropec", bufs=1, side="right")
            ropec = ropec_cm.__enter__()
            psp_cm = tc.tile_pool(name="ps_proj", bufs=8, space="PSUM")
            psp = psp_cm.__enter__()

            cos_sb = ropec.tile([HD, L], bf16, tag="cos")
            sinm_sb = ropec.tile([HD, L], bf16, tag="sinm")
            nc.scalar.dma_start(out=cos_sb[:], in_=cosd[:])
            nc.scalar.dma_start(out=sinm_sb[:], in_=sinmd[:])

            xqk_cm = tc.tile_pool(name="xqk", bufs=2, side="right")
            xqk = xqk_cm.__enter__()
            tpool_cm = tc.tile_pool(name="tpool", bufs=2, side="right")
            tpool = tpool_cm.__enter__()
            q_x0 = qk_phase(tc, wqd, QTa, wpool, xqk, tpool, psp, "q",
                            cos_sb, sinm_sb, gmajor_first=True)
            qk_phase(tc, wkd, KTa, wpool, xqk, tpool, psp, "k",
                     cos_sb, sinm_sb, delay_after=q_x0)
            tpool_cm.__exit__(None, None, None)
            xqk_cm.__exit__(None, None, None)
            ropec_cm.__exit__(None, None, None)
            wpool_cm.__exit__(None, None, None)
            psp_cm.__exit__(None, None, None)

            vp_cm = tc.tile_pool(name="vp", bufs=1, side="left")
            vp_outer = vp_cm.__enter__()
            Va = vp_outer.tile([128, NKB, HPC * HD], bf16, tag="va")
            # V-projection weights + x chunks + (later) Wo live on the
            # LEFT side in fresh regions: no WAR deps on the K-phase
            # pools, so their DMAs stream during K compute.  wv and wo
            # share one ring slot -- the wo load automatically waits for
            # the last V matmul to read wv.
            wvx_cm = tc.tile_pool(name="wvx", bufs=1, side="left")
            wvx = wvx_cm.__enter__()

            # -------- phases B + C (V projection interleaved) --------
            _attn_and_out(tc, nc, kind, block_pat, QTa, KTa, Va,
                          cst, ones_col, wod, wvd, xd, yT,
                          fp32, bf16, EXP, NHC, wvx)
            wvx_cm.__exit__(None, None, None)
            vp_cm.__exit__(None, None, None)

    nc.compile()
    return nc


def _attn_and_out(tc, nc, kind, block_pat, QTa, KTa, Va, cst, ones_col,
                  wod, wvd, xd, yT, fp32, bf16, EXP, NHC, wvx):
    ones_sb = cst[:, ones_col:ones_col + 1]
    with tc.tile_pool(name="otp", bufs=1, side="left") as otp:
        OTa = otp.tile([HD, HPC, L], bf16, tag="ota")

        # The attention phase is ACT(exp)-throughput-bound: per 128x512
        # block the exp costs ~0.68us while the S+O matmuls only cost
        # ~0.42us of PE.  So attention runs PAIR-major and PE filler is
        # kept next to every exp burst: V-projection part 2 (kp blocks
        # 8..15, only needed by pair 2,3) interleaves into pair (0,1);
        # out-projection tiles for q-chunks 0,1 interleave into pair
        # (2,3).
        #
        # q-chunk PAIRS inside the kp-block loop: S (and O) matmuls for
        # the two chunks sit back-to-back with the same stationary
        # operand (K block / V block), so the weight load amortizes
        # across both.  Softmax denominators: P blocks are elementwise-
        # accumulated into a per-strip D tile on the vector engine, and
        # a single ones-matmul per strip reduces D across partitions.
        with tc.tile_pool(name="pp", bufs=6, side="right") as ppool, \
             tc.tile_pool(name="rr", bufs=2, side="right") as rpool, \
             tc.tile_pool(name="bb", bufs=4, side="right") as bpool, \
             tc.tile_pool(name="dd", bufs=3, side="right") as dpool, \
             tc.tile_pool(name="ysb", bufs=3, side="right") as ypool, \
             tc.tile_pool(name="ps_s", bufs=3, space="PSUM") as ps_s, \
             tc.tile_pool(name="ps_o", bufs=2, space="PSUM") as ps_o, \
             tc.tile_pool(name="ps_r", bufs=1, space="PSUM") as ps_r:

            wv_sb = wvx.tile([128, NHC, HPC * HD], bf16, tag="big",
                             name="wv")
            for g in range(4):
                nc.sync.dma_start(out=wv_sb[:, 4 * g:4 * g + 4, :],
                                  in_=wvd[:, 4 * g:4 * g + 4, :])

            def emit_v_chunk(j):
                # V pos-major projection for x chunk j (1 kp block)
                x_sb = wvx.tile([128, NHC, VC], bf16, tag=f"xv{j % 2}",
                                name=f"xv{j}")
                nc.sync.dma_start(
                    out=x_sb[:], in_=xd[:, :, j * VC:(j + 1) * VC])
                psd = [ps_s.tile([128, QC], fp32, tag="pss",
                                 name=f"psv{j}_{dc}")
                       for dc in range(2)]
                for g in range(4):
                    for hc in range(4 * g, 4 * g + 4):
                        for dc in range(2):
                            nc.tensor.matmul(
                                psd[dc][:],
                                x_sb[:, hc, :],
                                wv_sb[:, hc, dc * QC:(dc + 1) * QC],
                                start=(hc == 0), stop=(hc == NHC - 1))
                for dc in range(2):
                    nc.scalar.copy(
                        Va[:, j, dc * QC:(dc + 1) * QC],
                        psd[dc][:])

            def emit_attn_head(h, jpair):
                blocks_j = {}
                first_i = {}
                last_i = {}
                for j in jpair:
                    for i in range(NKB):
                        live = [t for t in range(4)
                                if kind[4 * j + t][i] != 0]
                        if live:
                            blocks_j.setdefault(i, []).append((j, live))
                            if j not in first_i:
                                first_i[j] = i
                            last_i[j] = i
                if not first_i:
                    return
                pso = {j: ps_o.tile([128, QC], fp32, tag=f"pso{j % 2}",
                                    name=f"pso{h}_{j}")
                       for j in first_i}
                psr = {j: ps_r.tile([1, QC], fp32, tag="psr",
                                    name=f"psr{h}_{j}")
                       for j in first_i}
                dsb = {j: dpool.tile([128, QC], bf16, tag="d",
                                     name=f"d{h}_{j}")
                       for j in first_i}

                def emit_s(i, j, live):
                    t0, t1 = live[0], live[-1]
                    w0, w1 = t0 * 128, (t1 + 1) * 128
                    pss = ps_s.tile([128, QC], fp32, tag="pss",
                                    name=f"pss{h}_{j}_{i}")
                    nc.tensor.matmul(
                        pss[:, w0:w1],
                        KTa[:, h, i * 128:(i + 1) * 128],
                        QTa[:, h, j * QC + w0:j * QC + w1],
                        start=True, stop=True)
                    P = ppool.tile([128, QC], bf16, tag="p",
                                   name=f"p{h}_{j}_{i}")
                    first = (first_i[j] == i)
                    if w0 > 0 and first:
                        nc.vector.memset(P[:, 0:w0], 0.0)
                    if w1 < QC and first:
                        nc.vector.memset(P[:, w1:QC], 0.0)
                    nc.scalar.activation(P[:, w0:w1], pss[:, w0:w1],
                                         EXP, scale=SCALE)
                    for t in range(t0, t1 + 1):
                        qb = 4 * j + t
                        if kind[qb][i] == 0:
                            nc.vector.memset(
                                P[:, t * 128:(t + 1) * 128], 0.0)
                        elif kind[qb][i] == 2:
                            pat = block_pat[(qb, i)]
                            nc.vector.tensor_mul(
                                P[:, t * 128:(t + 1) * 128],
                                P[:, t * 128:(t + 1) * 128],
                                cst[:, pat * 128:(pat + 1) * 128])
                    # accumulate the softmax denominator contribution
                    # (vector engine; full width at first so D is fully
                    # initialized)
                    if first:
                        nc.vector.tensor_copy(dsb[j][:], P[:])
                    else:
                        nc.vector.tensor_add(dsb[j][:, w0:QC],
                                             dsb[j][:, w0:QC],
                                             P[:, w0:QC])
                    return (j, P, w0, first)

                def emit_ovr(i, group):
                    # O matmuls first (V stationary shared), then any
                    # strip-final denominator reduce + normalize
                    for j, P, w0, first in group:
                        m0 = 0 if first else w0
                        nc.tensor.matmul(
                            pso[j][:, m0:QC],
                            Va[:, i, h * HD:(h + 1) * HD],
                            P[:, m0:QC],
                            start=first, stop=(last_i[j] == i))
                    for j, P, w0, first in group:
                        if last_i[j] != i:
                            continue
                        nc.tensor.matmul(
                            psr[j][0:1, :], ones_sb, dsb[j][:],
                            start=True, stop=True)
                        r_sb = rpool.tile([128, QC], fp32, tag="r",
                                          name=f"r{h}_{j}")
                        nc.vector.reciprocal_approx_fast(
                            out=r_sb[0:1, :], in_=psr[j][0:1, :])
                        rb_sb = rpool.tile([128, QC], bf16, tag="rb",
                                           name=f"rb{h}_{j}")
                        nc.vector.tensor_copy(rb_sb[0:1, :],
                                              r_sb[0:1, :])
                        bc_sb = bpool.tile([128, QC], bf16, tag="bc",
                                           name=f"bc{h}_{j}")
                        nc.gpsimd.partition_broadcast(bc_sb[:],
                                                      rb_sb[0:1, :])
                        nc.vector.tensor_mul(
                            OTa[:, h, j * QC:(j + 1) * QC],
                            pso[j][:], bc_sb[:])

                prev = None
                for i in sorted(blocks_j):
                    cur = (i, [emit_s(i, j, live)
                               for j, live in blocks_j[i]])
                    if prev is not None:
                        emit_ovr(*prev)
                    prev = cur
                if prev is not None:
                    emit_ovr(*prev)

            def emit_c_tile(wo_sb, j, oc):
                # out-proj tile; PSUM comes from the shared ps_s ring
                ps = ps_s.tile([128, QC], fp32, tag="pss",
                               name=f"psc{j}_{oc}")
                for fc in range(HPC):
                    nc.tensor.matmul(
                        ps[:],
                        wo_sb[:, fc, oc * 128:(oc + 1) * 128],
                        OTa[:, fc, j * QC:(j + 1) * QC],
                        start=(fc == 0), stop=(fc == HPC - 1))
                y_sb = ypool.tile([128, QC], fp32, tag="y")
                nc.vector.tensor_copy(y_sb[:], ps[:])
                nc.sync.dma_start(
                    out=yT[oc * 128:(oc + 1) * 128,
                           j * QC:(j + 1) * QC],
                    in_=y_sb[:])

            # V part 1 (kp blocks 0..7) -- needed by pair (0,1)
            for vj in range(8):
                emit_v_chunk(vj)
            # pair (0,1) with V part 2 interleaved as PE filler
            for h in range(HPC):
                emit_attn_head(h, (0, 1))
                emit_v_chunk(8 + h)
            # Wo prefetch reuses wv's ring slot (WAR on the last V2
            # matmul readers; needed ~17us into pair (2,3) by the first
            # out-proj tile)
            wo_sb = wvx.tile([128, HPC, H], bf16, tag="big", name="wo")
            nc.sync.dma_start(out=wo_sb[:], in_=wod[:])
            # pair (2,3) with out-proj tiles for q-chunks 0,1
            # interleaved as PE filler
            ctiles01 = [(j, oc) for j in (0, 1)
                        for oc in range(H // 128)]
            for h in range(HPC):
                emit_attn_head(h, (2, 3))
                for (j, oc) in ctiles01[4 * h:4 * h + 4]:
                    emit_c_tile(wo_sb, j, oc)
            for j in (2, 3):
                for oc in range(H // 128):
                    emit_c_tile(wo_sb, j, oc)


def _prep_inputs(x, mask, Wq, Wk, Wv, Wo, patterns):
    import ml_dtypes
    bf16 = ml_dtypes.bfloat16

    # RoPE tables, d-major [HD, L]
    inv_freq = 1.0 / (ROPE_BASE ** (np.arange(0, HD, 2, dtype=np.float64)
                                    / HD))
    t = np.arange(L, dtype=np.float64)
    freqs = np.outer(t, inv_freq)                     # [L, HD/2]
    emb = np.concatenate((freqs, freqs), axis=-1)     # [L, HD]
    cos = np.cos(emb).T.astype(np.float32)            # [HD, L]
    sin = np.sin(emb).T.astype(np.float32)
    sinm = sin.copy()
    sinm[0:64] = -sin[0:64]
    cos_b = cos.astype(bf16)
    sinm_b = sinm.astype(bf16)

    npat = max(len(patterns), 1)
    maskd = np.zeros((npat, 128, 128), dtype=bf16)
    for i, p in enumerate(patterns):
        maskd[i] = p.astype(np.float32).astype(bf16)

    def pmajor(a2d):
        # [R, M] -> partition-major [128, R//128, M] with partition p
        # holding rows {p, 128+p, ...}; dense per-partition DMA layout
        r, m = a2d.shape
        return np.ascontiguousarray(
            a2d.reshape(r // 128, 128, m).transpose(1, 0, 2))

    in_maps = []
    for c in range(NCORES):
        b, half = c // 2, c % 2
        rows = slice(half * HPC * HD, (half + 1) * HPC * HD)
        in_maps.append({
            "xd": pmajor(np.ascontiguousarray(x[b].T).astype(bf16)),
            "wqd": pmajor(np.ascontiguousarray(Wq[rows, :].T).astype(bf16)),
            "wkd": pmajor(np.ascontiguousarray(Wk[rows, :].T).astype(bf16)),
            "wvd": pmajor(np.ascontiguousarray(Wv[rows, :].T).astype(bf16)),
            "wod": pmajor(np.ascontiguousarray(Wo[:, rows].T).astype(bf16)),
            "cosd": cos_b,
            "sinmd": sinm_b,
            "maskd": maskd,
        })
    return in_maps


def kernel(x, mask, Wq, Wk, Wv, Wo, _trace=False):
    from concourse.bass_utils import run_bass_kernel_spmd

    x = np.asarray(x, dtype=np.float32)
    mask2d = np.asarray(mask, dtype=np.int32).reshape(L, L)
    key = mask2d.tobytes()
    if key not in _cache:
        kind, patterns, block_pat = _analyze_mask(mask2d)
        nc = _build(kind, block_pat, len(patterns))
        _cache[key] = (nc, patterns)
    nc, patterns = _cache[key]

    in_maps = _prep_inputs(x, mask, np.asarray(Wq, np.float32),
                           np.asarray(Wk, np.float32),
                           np.asarray(Wv, np.float32),
                           np.asarray(Wo, np.float32), patterns)
    res = run_bass_kernel_spmd(nc, in_maps, list(range(NCORES)),
                               trace=_trace)
    y = np.empty((B, L, H), dtype=np.float32)
    for b in range(B):
        acc = res.results[2 * b]["yT"].astype(np.float32) + \
              res.results[2 * b + 1]["yT"].astype(np.float32)
        y[b] = acc.T
    if _trace:
        kernel.last_results = res
    return y


if __name__ == "__main__":
    import reference
    inputs = reference.setup_inputs()
    inputs = {k: np.asarray(v) for k, v in inputs.items()}
    out = kernel(**inputs)
    exp = np.asarray(reference.reference(**{k: v for k, v in inputs.items()}))
    err = np.abs(out - exp).max() / np.abs(exp).max()
    print("rel err (absmax):", err)
